# revision 5
# baseline (speedup 1.0000x reference)
"""Trainium2 Bass kernel for nn_Concat_84653805404632.

Reference computation: x is [70, 128, 512] f32; rows 0..19 are supports
(ns_all = n_class*n_support = 20), rows 20..69 are queries (nq_all = 50).
Output [1000, 128, 1024] where out[q*20+s] = concat(sup[s], qry[q], axis=-1).

Pure data movement (memory regime). Sharding: the (query, support) pair grid
[50 x 20] is split as (2 query-halves) x (4 support-fifths) -> 8 cores, each
producing exactly 125 output rows (64 MB) with an identical SPMD access
pattern.

Per core: the 5 support tiles are DMA-loaded directly into the sup columns of
two interleaved "image" buffers in SBUF; the VectorEngine broadcasts each
query tile into the qry columns (SBUF engine ports are separate from the DMA
AXI ports, so this is free); each query then leaves as ONE 2.62 MB write DMA
whose descriptors are full 4 KB rows. 4 KB descriptors matter: SDMA engine 15
has ~15 ns/packet extra fixed cost and the in-order descriptor generator
stalls on its ring, so the whole core runs at engine-15's packet rate —
bigger packets raise that ceiling from ~325 GB/s to ~390 GB/s.
"""

import os
import sys

import numpy as np

for _p in ("/opt/trn_rl_repo", "/root/.axon_site/_ro/trn_rl_repo"):
    if os.path.isdir(_p) and _p not in sys.path:
        sys.path.insert(0, _p)

import concourse.bass as bass
import concourse.mybir as mybir
from concourse.bass_utils import run_bass_kernel_spmd

NS_ALL = 20  # n_class * n_support
NQ_ALL = 50  # n_class * n_query
D = 128
F = 512
QH = 25  # queries per core  (NQ_ALL / 2)
SF = 5  # supports per core (NS_ALL / 4)
QCH = 5  # query tiles per load chunk
PKEEP = 127  # partitions written from SBUF; column 127 spilled from DRAM
N_CORES = 8

_NC_CACHE = None


def _build_nc():
    nc = bass.Bass()
    sup = nc.declare_dram_parameter("sup", [SF, D, F], mybir.dt.float32, isOutput=False)
    qry = nc.declare_dram_parameter("qry", [QH, D, F], mybir.dt.float32, isOutput=False)
    out = nc.declare_dram_parameter(
        "out", [QH * SF, D, 2 * F], mybir.dt.float32, isOutput=True
    )

    with (
        nc.sbuf_tensor([D, QH * F], mybir.dt.float32) as qry_t,
        nc.sbuf_tensor([D, SF * 2 * F], mybir.dt.float32) as img0,
        nc.sbuf_tensor([D, SF * 2 * F], mybir.dt.float32) as img1,
        nc.semaphore("img_sup_sem0") as img_sup_sem0,
        nc.semaphore("qry_sem0") as qry_sem0,
        nc.semaphore("qry_sem1") as qry_sem1,
        nc.semaphore("qry_sem2") as qry_sem2,
        nc.semaphore("qry_sem3") as qry_sem3,
        nc.semaphore("qry_sem4") as qry_sem4,
        nc.semaphore("spill_sem") as spill_sem,
        nc.semaphore("dve_sem") as dve_sem,
        nc.semaphore("out_sem0") as out_sem0,
        nc.semaphore("out_sem1") as out_sem1,
        nc.Block() as block,
    ):
        imgs = [img0, img1]
        qry_sems = [qry_sem0, qry_sem1, qry_sem2, qry_sem3, qry_sem4]
        out_sems = [out_sem0, out_sem1]

        def img_view(b):
            return imgs[b][:].rearrange("p (s f2) -> p s f2", f2=2 * F)

        @block.sync
        def _(sync):
            # sup tiles into img0's sup columns; DVE mirrors them to img1.
            sync.dma_start(
                img_view(0)[:, :, 0:F], sup[:].transpose([1, 0, 2])
            ).then_inc(img_sup_sem0, 16)
            sync.dma_start(
                qry_t[:, 0 : QCH * F],
                qry[0:QCH].transpose([1, 0, 2]),
            ).then_inc(qry_sems[0], 16)
            for c in range(1, QH // QCH):
                sync.dma_start(
                    qry_t[:, QCH * F * c : QCH * F * (c + 1)],
                    qry[QCH * c : QCH * (c + 1)].transpose([1, 0, 2]),
                ).then_inc(qry_sems[c], 16)

        @block.gpsimd
        def _(gpsimd):
            # Column-127 spill: write the excluded column for ALL output rows
            # straight from DRAM inputs with broadcast (stride-0) sources.
            # On the gpsimd (SWDGE) queue so it drains alongside the HWDGE
            # rings; the sync-ring version crashed NRT.
            sup_spill_src = (
                sup[0:SF, 127, 0:F].unsqueeze(0).broadcast_to([QH, SF, F])
            )
            gpsimd.dma_start(out[:, 127, 0:F], sup_spill_src).then_inc(
                spill_sem, 16
            )
            qry_spill_src = (
                qry[0:QH, 127, 0:F].unsqueeze(1).broadcast_to([QH, SF, F])
            )
            gpsimd.dma_start(out[:, 127, F : 2 * F], qry_spill_src).then_inc(
                spill_sem, 16
            )
            gpsimd.wait_ge(spill_sem, 32)

        @block.vector
        def _(vector):
            # op 1: mirror sup columns img0 -> img1
            vector.wait_ge(img_sup_sem0, 16)
            vector.tensor_copy(
                img_view(1)[:, :, 0:F], img_view(0)[:, :, 0:F]
            ).then_inc(dve_sem, 1)
            # ops 2..26: query broadcast into image q%2
            for q in range(QH):
                vector.wait_ge(qry_sems[q // QCH], 16)
                if q >= 2:
                    vector.wait_ge(out_sems[q % 2], 16 * (q // 2))
                dst = img_view(q % 2)[:, :, F : 2 * F]
                src = (
                    qry_t[:, F * q : F * (q + 1)]
                    .unsqueeze(1)
                    .broadcast_to([D, SF, F])
                )
                vector.tensor_copy(dst, src).then_inc(dve_sem, 1)

        @block.scalar
        def _(scalar):
            for q in range(QH):
                if q == 0:
                    scalar.wait_ge(img_sup_sem0, 16)
                scalar.wait_ge(dve_sem, q + 2)
                dst = out[SF * q : SF * (q + 1), :, :].transpose([1, 0, 2])[0:PKEEP]
                scalar.dma_start(dst, imgs[q % 2][0:PKEEP, :]).then_inc(
                    out_sems[q % 2], 16
                )
            scalar.wait_ge(out_sem0, 16 * ((QH + 1) // 2))
            scalar.wait_ge(out_sem1, 16 * (QH // 2))

    return nc


def _get_nc():
    global _NC_CACHE
    if _NC_CACHE is None:
        _NC_CACHE = _build_nc()
    return _NC_CACHE


def kernel(**inputs) -> np.ndarray:
    x = np.ascontiguousarray(np.asarray(inputs["x"], dtype=np.float32))
    assert x.shape == (NS_ALL + NQ_ALL, D, F), x.shape

    sup_all = x[:NS_ALL]
    qry_all = x[NS_ALL:]

    in_maps = []
    for k in range(N_CORES):
        h, f = divmod(k, 4)
        in_maps.append(
            {
                "sup": np.ascontiguousarray(sup_all[SF * f : SF * (f + 1)]),
                "qry": np.ascontiguousarray(qry_all[QH * h : QH * (h + 1)]),
            }
        )

    nc = _get_nc()
    res = run_bass_kernel_spmd(nc, in_maps, core_ids=list(range(N_CORES)))

    full = np.empty((NQ_ALL, NS_ALL, D, 2 * F), dtype=np.float32)
    for k in range(N_CORES):
        h, f = divmod(k, 4)
        out_k = np.asarray(res.results[k]["out"]).reshape(QH, SF, D, 2 * F)
        full[QH * h : QH * (h + 1), SF * f : SF * (f + 1)] = out_k
    return full.reshape(NQ_ALL * NS_ALL, D, 2 * F)
